# revision 24
# baseline (speedup 1.0000x reference)
"""Trainium2 Bass kernel for nn_ContrastiveLearning (NT-Xent over pairwise
symmetrized-KL of diagonal Gaussians).

Math (equivalent reformulation of the reference):
  loss[i,j] = -0.25*(A[i,j] + A[j,i] + md[i,j] + md[j,i] - 2D)   (ld terms cancel)
  A[i,j]+A[j,i] + md[i,j]+md[j,i] - (q_i + q_j)
      = <u_i,s_j> + <s_i,u_j> - 2<w_i,m_j> - 2<m_i,w_j>  =: total[i,j]
  with u=1/sigma, s=sigma+mu^2, w=mu/sigma, m=mu, q_x=<mu^2,1/sigma>_x.
  Row-constant factors cancel in lf_i = -log2(num_i)+log2(den_i)+log2(cnt_i),
  so the device computes E[i,j] = exp(-0.25*(total[i,j] + q_j + pen_j) + C)
  where pen_j = BIG kills padding columns, the diagonal is killed by adding
  BIG*I to total via an extra matmul, and C keeps fp32 exp in range.
  num_i = sum_j E*[lab_i==lab_j],  den_i = sum_j E.
  cnt, log2 and the final masked mean are O(N) host work.

Device layout (v2):
  - Host compacts valid tokens, pads to a multiple of 384, applies ReLU and
    transposes: h^T [768, n_pad] is DMA'd feature-major (no device transposes).
  - Linear biases are folded into the matmul as rank-1 terms; the ELU/sigma
    chain runs on DVE + one Exp on the Scalar engine (GPSIMD unused).
  - lab_i==lab_j is a rank-16 one-hot matmul (PE); num row-sums come from a
    fused DVE tensor_tensor_reduce; den row-sums from the Exp's accumulator.
  - Anchor rows are data-parallel over 8 cores via a host-side roll of the
    token axis, so a single SPMD program uses static addressing.
"""
import numpy as np

EMBED_DIM = 128
H_DIM = 768
C_SHIFT = 40.0           # global exponent shift, cancels between num and den
BIG = 1024.0 * 1024.0    # diag/pad kill: exp(-0.25*BIG) == 0 in fp32
SQRT_BIG = 1024.0
GRP = 384                # token-group / j-tile width (>=256 keeps f32r matmul fast)
NCLS = 16                # one-hot label partitions (labels are 0..9)

_cache = {}


def _build(n_pad):
    import concourse.bass as bass
    import concourse.tile as tile
    from concourse import bacc, mybir
    from concourse.tile_rust import add_dep_helper

    f32 = mybir.dt.float32
    f32r = mybir.dt.float32r
    AF = mybir.ActivationFunctionType
    ALU = mybir.AluOpType
    AX = mybir.AxisListType

    n_groups = n_pad // GRP
    n_anchor = n_pad // 8
    assert n_anchor <= GRP, "anchors must fit in group 0"
    it_ws = [128] * (n_anchor // 128)
    if n_anchor % 128:
        it_ws.append(n_anchor % 128)
    n_it = len(it_ws)

    nc = bacc.Bacc(None, target_bir_lowering=False, debug=False)
    ht_d = nc.declare_dram_parameter("ht", [H_DIM, n_pad], f32, isOutput=False)
    wmu_d = nc.declare_dram_parameter("wmu", [H_DIM, EMBED_DIM], f32, isOutput=False)
    wsig_d = nc.declare_dram_parameter("wsig", [H_DIM, EMBED_DIM], f32, isOutput=False)
    bmu_d = nc.declare_dram_parameter("bmu", [EMBED_DIM], f32, isOutput=False)
    bsig_d = nc.declare_dram_parameter("bsig", [EMBED_DIM], f32, isOutput=False)
    pen_d = nc.declare_dram_parameter("pen", [n_pad], f32, isOutput=False)
    rowoh_d = nc.declare_dram_parameter("rowoh", [NCLS + 1, n_anchor], f32, isOutput=False)
    coloh_d = nc.declare_dram_parameter("coloh", [NCLS + 1, n_pad], f32, isOutput=False)
    eqd_d = nc.declare_dram_parameter("eqd", [128, 128], f32, isOutput=False)
    out_d = nc.declare_dram_parameter("out", [n_anchor, 2], f32, isOutput=True)

    with tile.TileContext(nc) as tc:
        with tc.tile_pool(name="const", bufs=1) as const, \
             tc.tile_pool(name="feat", bufs=1) as feat, \
             tc.tile_pool(name="work", bufs=2) as work, \
             tc.tile_pool(name="eph", bufs=3) as eph, \
             tc.tile_pool(name="acc", bufs=1) as accp, \
             tc.tile_pool(name="psMZ", bufs=2, space="PSUM") as psMZ, \
             tc.tile_pool(name="psP", bufs=3, space="PSUM") as psP, \
             tc.tile_pool(name="psQ", bufs=1, space="PSUM") as psQ:

            # ---------------- input DMAs (ordered for overlap) ----------------
            wmu_t = const.tile([128, 6, 128], f32r)
            wsig_t = const.tile([128, 6, 128], f32r)
            nc.sync.dma_start(wmu_t[:], wmu_d.rearrange("(a b) c -> b a c", b=128).bitcast(f32r))
            nc.sync.dma_start(wsig_t[:], wsig_d.rearrange("(a b) c -> b a c", b=128).bitcast(f32r))
            bmuc_t = const.tile([128, 1], f32)
            bsigr_t = const.tile([1, 128], f32r)
            nc.sync.dma_start(bmuc_t[:], bmu_d.rearrange("(p o) -> p o", o=1))
            nc.sync.dma_start(bsigr_t[:], bsig_d.rearrange("(o p) -> o p", o=1).bitcast(f32r))

            h_t = const.tile([128, 6, n_pad], f32r)
            for g in range(min(2, n_groups)):
                gs = slice(g * GRP, (g + 1) * GRP)
                for k in range(6):
                    nc.sync.dma_start(h_t[:, k, gs],
                                      ht_d[k * 128:(k + 1) * 128, gs].bitcast(f32r))
            eqd_r = const.tile([128, 128], f32r)
            nc.sync.dma_start(eqd_r[:], eqd_d[:].bitcast(f32r))
            pen_t = const.tile([1, n_pad], f32r)
            nc.sync.dma_start(pen_t[:], pen_d.rearrange("(o n) -> o n", o=1).bitcast(f32r))
            rowoh_t = const.tile([NCLS + 1, n_anchor], f32r)
            nc.sync.dma_start(rowoh_t[:], rowoh_d[:].bitcast(f32r))
            coloh_t = const.tile([NCLS + 1, n_pad], f32r)
            nc.sync.dma_start(coloh_t[:], coloh_d[:].bitcast(f32r))
            for g in range(2, n_groups):
                gs = slice(g * GRP, (g + 1) * GRP)
                for k in range(6):
                    nc.sync.dma_start(h_t[:, k, gs],
                                      ht_d[k * 128:(k + 1) * 128, gs].bitcast(f32r))

            # ---------------- small constants ----------------
            ones_r = const.tile([1, GRP], f32r)
            nc.vector.memset(ones_r[:].bitcast(f32), 1.0)
            onescol_r = const.tile([128, 1], f32r)
            nc.vector.memset(onescol_r[:].bitcast(f32), 1.0)
            cbias_t = const.tile([128, 1], f32)
            nc.vector.memset(cbias_t[:], C_SHIFT)

            # ---------------- persistent feature tensors ----------------
            m_f = feat.tile([128, n_pad], f32r)      # mu            (feature-major)
            u_f = feat.tile([128, n_pad], f32r)      # 1/sigma
            s_f = feat.tile([128, n_pad], f32r)      # sigma + mu^2
            w_f = feat.tile([128, n_pad], f32r)      # mu/sigma
            qcol = feat.tile([1, n_pad], f32r)       # q_j + pen_j
            w2a = feat.tile([128, n_anchor], f32r)   # -2*mu/sigma (anchors)
            m2a = feat.tile([128, n_anchor], f32r)   # -2*mu       (anchors)
            num_sl = accp.tile([128, n_it, n_groups], f32)
            den_sl = accp.tile([128, n_it, n_groups], f32)
            nd = accp.tile([128, n_it, 2], f32)

            pu_g = [None] * n_groups                 # mu^2/sigma, consumed by q matmul

            def phase1(g):
                gs = slice(g * GRP, (g + 1) * GRP)
                ps_mu = psMZ.tile([128, GRP], f32, tag="mu")
                ps_z = psMZ.tile([128, GRP], f32, tag="z")
                # sigma-head bias as a rank-1 matmul term; mu/z interleaved per
                # k-chunk so PE streams as h chunks arrive from DMA
                nc.tensor.matmul(ps_z[:], bsigr_t[:], ones_r[:], start=True, stop=False)
                for k in range(6):
                    nc.tensor.matmul(ps_mu[:], wmu_t[:, k, :], h_t[:, k, gs],
                                     start=(k == 0), stop=(k == 5))
                    nc.tensor.matmul(ps_z[:], wsig_t[:, k, :], h_t[:, k, gs],
                                     start=False, stop=(k == 5))

                nc.vector.tensor_scalar_add(m_f[:, gs], ps_mu[:], bmuc_t[:])
                zm = work.tile([128, GRP], f32, tag="zm")
                nc.vector.tensor_scalar_min(zm[:], ps_z[:], 0.0)
                e1 = work.tile([128, GRP], f32, tag="e1")
                nc.scalar.activation(e1[:], zm[:], AF.Exp)
                # sigma = max(z,0) + exp(min(z,0))   (elu(z)+1)
                sig = work.tile([128, GRP], f32, tag="sig")
                nc.vector.scalar_tensor_tensor(sig[:], ps_z[:], 0.0, e1[:],
                                               ALU.max, ALU.add)
                ls = work.tile([128, GRP], f32, tag="ls")
                nc.scalar.activation(ls[:], sig[:], AF.Ln)
                nc.scalar.activation(u_f[:, gs], ls[:], AF.Exp, scale=-1.0)
                psq = work.tile([128, GRP], f32, tag="psq")
                nc.vector.tensor_mul(psq[:], m_f[:, gs].bitcast(f32),
                                     m_f[:, gs].bitcast(f32))
                nc.vector.tensor_add(s_f[:, gs], psq[:], sig[:])
                nc.vector.tensor_mul(w_f[:, gs], m_f[:, gs].bitcast(f32),
                                     u_f[:, gs].bitcast(f32))
                pu = work.tile([128, GRP], f32r, tag="pu")
                nc.vector.tensor_mul(pu[:], psq[:], u_f[:, gs].bitcast(f32))
                pu_g[g] = pu

            def qmm(g):
                gs = slice(g * GRP, (g + 1) * GRP)
                ps_q = psQ.tile([1, GRP], f32, tag="q")
                nc.tensor.matmul(ps_q[:], onescol_r[:], pu_g[g][:], start=True, stop=True)
                nc.vector.tensor_add(qcol[:, gs], ps_q[:], pen_t[:, gs].bitcast(f32))

            def anchors():
                nc.vector.tensor_scalar_mul(w2a[:], w_f[:, 0:n_anchor].bitcast(f32), -2.0)
                nc.vector.tensor_scalar_mul(m2a[:], m_f[:, 0:n_anchor].bitcast(f32), -2.0)

            def ph2(jt):
                jsl = slice(jt * GRP, (jt + 1) * GRP)
                ps2s = []
                exp1s = []
                for it, w in enumerate(it_ws):
                    isl = slice(it * 128, it * 128 + w)
                    ps2 = psP.tile([128, GRP], f32, tag="p2")
                    ps2s.append(ps2)
                    nc.tensor.matmul(ps2[:w], s_f[:, isl], u_f[:, jsl],
                                     start=True, stop=False)
                    nc.tensor.matmul(ps2[:w], u_f[:, isl], s_f[:, jsl],
                                     start=False, stop=False)
                    nc.tensor.matmul(ps2[:w], w2a[:, isl], m_f[:, jsl],
                                     start=False, stop=False)
                    nc.tensor.matmul(ps2[:w], m2a[:, isl], w_f[:, jsl],
                                     start=False, stop=False)
                    diag = (jt == 0)
                    nc.tensor.matmul(ps2[:w], ones_r[:, 0:w], qcol[:, jsl],
                                     start=False, stop=not diag)
                    if diag:
                        doff = it * 128
                        nc.tensor.matmul(ps2[:w, doff:doff + w],
                                         eqd_r[:, 0:w], eqd_r[:, 0:w],
                                         start=False, stop=True)
                    e_t = eph.tile([128, GRP], f32, tag="E")
                    exp1 = nc.scalar.activation(e_t[:w], ps2[:w], AF.Exp, scale=-0.25,
                                                bias=cbias_t[:w],
                                                accum_out=den_sl[:w, it, jt:jt + 1])
                    exp1s.append(exp1)
                # second pass: add -4*BIG*(1-labq) into ps2, exp again -> num
                for it, w in enumerate(it_ws):
                    isl = slice(it * 128, it * 128 + w)
                    ps2 = ps2s[it]
                    pen_mm = nc.tensor.matmul(ps2[:w], rowoh_t[:, isl], coloh_t[:, jsl],
                                              start=False, stop=True, skip_group_check=True)
                    add_dep_helper(pen_mm.ins, exp1s[it].ins, sync=True,
                                   reason="labq penalty matmul must follow den exp read")
                    e2 = eph.tile([128, GRP], f32, tag="E2")
                    nc.scalar.activation(e2[:w], ps2[:w], AF.Exp, scale=-0.25,
                                         bias=cbias_t[:w],
                                         accum_out=num_sl[:w, it, jt:jt + 1])

            phase1(0)
            anchors()
            for g in range(1, n_groups):
                phase1(g)
                qmm(g - 1)
                ph2(g - 1)
            qmm(n_groups - 1)
            ph2(n_groups - 1)

            for it, w in enumerate(it_ws):
                nc.vector.tensor_reduce(nd[:w, it, 0:1], num_sl[:w, it, :], AX.X, ALU.add)
                nc.vector.tensor_reduce(nd[:w, it, 1:2], den_sl[:w, it, :], AX.X, ALU.add)
                nc.sync.dma_start(out_d[it * 128:it * 128 + w, :], nd[0:w, it, :])

    nc.compile()
    return nc


def _prepare(ent_embeddings, ent_type_ids, ent_mask, W_mu, b_mu, W_sigma, b_sigma):
    emb = np.ascontiguousarray(np.asarray(ent_embeddings, dtype=np.float32)).reshape(-1, H_DIM)
    labels = np.asarray(ent_type_ids).reshape(-1).astype(np.int64)
    mask = np.asarray(ent_mask).reshape(-1).astype(np.int64)
    W_mu = np.ascontiguousarray(np.asarray(W_mu, dtype=np.float32))
    W_sigma = np.ascontiguousarray(np.asarray(W_sigma, dtype=np.float32))
    b_mu = np.ascontiguousarray(np.asarray(b_mu, dtype=np.float32))
    b_sigma = np.ascontiguousarray(np.asarray(b_sigma, dtype=np.float32))

    valid = (mask == 1) & (labels >= 0)
    vidx = np.nonzero(valid)[0]
    n_v = len(vidx)
    if n_v == 0:
        return None

    n_pad = GRP * max(1, -(-n_v // GRP))
    n_anchor = n_pad // 8
    labs = labels[vidx]
    assert labs.max() < NCLS, "labels exceed one-hot width"

    h = np.zeros((n_pad, H_DIM), dtype=np.float32)
    h[:n_v] = np.maximum(emb[vidx], 0.0)
    hT = np.ascontiguousarray(h.T)                      # [768, n_pad]
    labc = np.full(n_pad, -1, dtype=np.int64)
    labc[:n_v] = labs
    pen = np.full(n_pad, BIG, dtype=np.float32)
    pen[:n_v] = 0.0
    eqd = (np.eye(128, dtype=np.float32) * SQRT_BIG)
    cls = np.arange(NCLS, dtype=np.int64)

    if n_pad not in _cache:
        _cache[n_pad] = _build(n_pad)
    nc = _cache[n_pad]

    in_maps = []
    for c in range(8):
        r = c * n_anchor
        labr = np.roll(labc, -r)
        # penalty matmul operands: sum_c rowP[c,i]*colP[c,j] = 4*BIG*(1-labq)
        rowP = np.empty((NCLS + 1, n_anchor), dtype=np.float32)
        rowP[:NCLS] = -2.0 * SQRT_BIG * (labr[None, :n_anchor] == cls[:, None])
        rowP[NCLS] = 2.0 * SQRT_BIG
        colP = np.empty((NCLS + 1, n_pad), dtype=np.float32)
        colP[:NCLS] = 2.0 * SQRT_BIG * (labr[None, :] == cls[:, None])
        colP[NCLS] = 2.0 * SQRT_BIG
        in_maps.append({
            "ht": np.ascontiguousarray(np.roll(hT, -r, axis=1)),
            "wmu": W_mu, "wsig": W_sigma, "bmu": b_mu, "bsig": b_sigma,
            "pen": np.roll(pen, -r),
            "rowoh": rowP,
            "coloh": colP,
            "eqd": eqd,
        })
    return dict(nc=nc, in_maps=in_maps, n_pad=n_pad, n_anchor=n_anchor,
                n_v=n_v, labs=labs)


def _finish(prep, res):
    n_pad, n_anchor, n_v = prep["n_pad"], prep["n_anchor"], prep["n_v"]
    num = np.empty(n_pad, dtype=np.float32)
    den = np.empty(n_pad, dtype=np.float32)
    for c in range(8):
        nd = res.results[c]["out"]
        rows = (np.arange(n_anchor) + c * n_anchor) % n_pad
        num[rows] = nd[:, 0]
        den[rows] = nd[:, 1]

    labs = prep["labs"]
    hist = np.bincount(labs, minlength=int(labs.max()) + 1)
    cnt = (hist[labs] - 1).astype(np.float64)
    sel = cnt > 0
    n_sel = max(sel.sum(), 1)
    num_v = num[:n_v].astype(np.float64)
    den_v = den[:n_v].astype(np.float64)
    safe_num = np.where(sel, num_v, 1.0)
    safe_den = np.where(sel, den_v, 1.0)
    safe_cnt = np.where(sel, cnt, 1.0)
    lf = (np.log(safe_den) - np.log(safe_num)) / np.log(2.0) + np.log2(safe_cnt)
    total = np.sum(np.where(sel, lf, 0.0)) / n_sel
    return np.float32(total)


def kernel(ent_embeddings, ent_type_ids, ent_mask, W_mu, b_mu, W_sigma, b_sigma):
    from concourse.bass_utils import run_bass_kernel_spmd

    prep = _prepare(ent_embeddings, ent_type_ids, ent_mask,
                    W_mu, b_mu, W_sigma, b_sigma)
    if prep is None:
        return np.float32(0.0)
    res = run_bass_kernel_spmd(prep["nc"], prep["in_maps"], list(range(8)))
    return _finish(prep, res)


# revision 31
# speedup vs baseline: 1.0385x; 1.0385x over previous
"""Trainium2 Bass kernel for nn_ContrastiveLearning (NT-Xent over pairwise
symmetrized-KL of diagonal Gaussians).

Math (equivalent reformulation of the reference):
  loss[i,j] = -0.25*(A[i,j] + A[j,i] + md[i,j] + md[j,i] - 2D)   (ld terms cancel)
  A[i,j]+A[j,i] + md[i,j]+md[j,i] - (q_i + q_j)
      = <u_i,s_j> + <s_i,u_j> - 2<w_i,m_j> - 2<m_i,w_j>  =: total[i,j]
  with u=1/sigma, s=sigma+mu^2, w=mu/sigma, m=mu, q_x=<mu^2,1/sigma>_x.
  Row-constant factors cancel in lf_i = -log2(num_i)+log2(den_i)+log2(cnt_i),
  so the device computes E[i,j] = exp(-0.25*(total[i,j] + q_j + pen_j) + C)
  where pen_j = BIG kills padding columns, the diagonal is killed by adding
  BIG*I to total via an extra matmul, and C keeps fp32 exp in range.
  num_i = sum_j E*[lab_i==lab_j],  den_i = sum_j E.
  cnt, log2 and the final masked mean are O(N) host work.

Device layout (v2):
  - Host compacts valid tokens, pads to a multiple of 384, applies ReLU and
    transposes: h^T [768, n_pad] is DMA'd feature-major (no device transposes).
  - Linear biases are folded into the matmul as rank-1 terms; the ELU/sigma
    chain runs on DVE + one Exp on the Scalar engine (GPSIMD unused).
  - lab_i==lab_j is a rank-16 one-hot matmul (PE); num row-sums come from a
    fused DVE tensor_tensor_reduce; den row-sums from the Exp's accumulator.
  - Anchor rows are data-parallel over 8 cores via a host-side roll of the
    token axis, so a single SPMD program uses static addressing.
"""
import numpy as np

EMBED_DIM = 128
H_DIM = 768
C_SHIFT = 40.0           # global exponent shift, cancels between num and den
BIG = 1024.0 * 1024.0    # diag/pad kill: exp(-0.25*BIG) == 0 in fp32
SQRT_BIG = 1024.0
GRP = 384                # token-group / j-tile width (>=256 keeps f32r matmul fast)
NCLS = 16                # one-hot label partitions (labels are 0..9)

_cache = {}


def _build(n_pad):
    import concourse.bass as bass
    import concourse.tile as tile
    from concourse import bacc, mybir
    from concourse.tile_rust import add_dep_helper

    f32 = mybir.dt.float32
    f32r = mybir.dt.float32r
    bf16 = mybir.dt.bfloat16
    AF = mybir.ActivationFunctionType
    ALU = mybir.AluOpType
    AX = mybir.AxisListType

    n_groups = n_pad // GRP
    n_anchor = n_pad // 8
    assert n_anchor <= GRP, "anchors must fit in group 0"
    it_ws = [128] * (n_anchor // 128)
    if n_anchor % 128:
        it_ws.append(n_anchor % 128)
    n_it = len(it_ws)

    nc = bacc.Bacc(None, target_bir_lowering=False, debug=False)
    ht_d = nc.declare_dram_parameter("ht", [H_DIM, n_pad], bf16, isOutput=False)
    wmu_d = nc.declare_dram_parameter("wmu", [H_DIM, EMBED_DIM], bf16, isOutput=False)
    wsig_d = nc.declare_dram_parameter("wsig", [H_DIM, EMBED_DIM], bf16, isOutput=False)
    bmu_d = nc.declare_dram_parameter("bmu", [EMBED_DIM], f32, isOutput=False)
    bsig_d = nc.declare_dram_parameter("bsig", [EMBED_DIM], f32, isOutput=False)
    pen_d = nc.declare_dram_parameter("pen", [n_pad], f32, isOutput=False)
    rowoh_d = nc.declare_dram_parameter("rowoh", [NCLS + 1, n_anchor], f32, isOutput=False)
    coloh_d = nc.declare_dram_parameter("coloh", [NCLS + 1, n_pad], f32, isOutput=False)
    eqd_d = nc.declare_dram_parameter("eqd", [128, 128], f32, isOutput=False)
    out_d = nc.declare_dram_parameter("out", [n_anchor, 2], f32, isOutput=True)

    with tile.TileContext(nc) as tc:
        with tc.tile_pool(name="const", bufs=1) as const, \
             tc.tile_pool(name="feat", bufs=1) as feat, \
             tc.tile_pool(name="work", bufs=2) as work, \
             tc.tile_pool(name="eph", bufs=3) as eph, \
             tc.tile_pool(name="acc", bufs=1) as accp, \
             tc.tile_pool(name="psMZ", bufs=2, space="PSUM") as psMZ, \
             tc.tile_pool(name="psP", bufs=3, space="PSUM") as psP, \
             tc.tile_pool(name="psQ", bufs=1, space="PSUM") as psQ:

            # ---------------- input DMAs (ordered for overlap) ----------------
            wmu_t = const.tile([128, 6, 128], bf16)
            wsig_t = const.tile([128, 6, 128], bf16)
            nc.sync.dma_start(wmu_t[:], wmu_d.rearrange("(a b) c -> b a c", b=128))
            nc.sync.dma_start(wsig_t[:], wsig_d.rearrange("(a b) c -> b a c", b=128))

            h_t = const.tile([128, 6, n_pad], bf16)
            ht_r = ht_d.rearrange("(a b) c -> b a c", b=128)
            for g in range(n_groups):
                gs = slice(g * GRP, (g + 1) * GRP)
                nc.sync.dma_start(h_t[:, :, gs], ht_r[:, :, gs])

            bmuc_t = const.tile([128, 1], f32)
            bsigr_t = const.tile([1, 128], f32r)
            nc.sync.dma_start(bmuc_t[:], bmu_d.rearrange("(p o) -> p o", o=1))
            nc.sync.dma_start(bsigr_t[:], bsig_d.rearrange("(o p) -> o p", o=1).bitcast(f32r))
            eqd_r = const.tile([128, 128], f32r)
            nc.sync.dma_start(eqd_r[:], eqd_d[:].bitcast(f32r))
            pen_t = const.tile([1, n_pad], f32r)
            nc.sync.dma_start(pen_t[:], pen_d.rearrange("(o n) -> o n", o=1).bitcast(f32r))
            rowoh_t = const.tile([NCLS + 1, n_anchor], f32r)
            nc.sync.dma_start(rowoh_t[:], rowoh_d[:].bitcast(f32r))
            coloh_t = const.tile([NCLS + 1, n_pad], f32r)
            nc.sync.dma_start(coloh_t[:], coloh_d[:].bitcast(f32r))

            # ---------------- small constants ----------------
            ones_r = const.tile([1, GRP], f32r)
            nc.vector.memset(ones_r[:].bitcast(f32), 1.0)
            onescol_r = const.tile([128, 1], f32r)
            nc.vector.memset(onescol_r[:].bitcast(f32), 1.0)
            cbias_t = const.tile([128, 1], f32)
            nc.vector.memset(cbias_t[:], C_SHIFT)

            # ---------------- persistent feature tensors ----------------
            m_f = feat.tile([128, n_pad], f32r)      # mu            (feature-major)
            u_f = feat.tile([128, n_pad], f32r)      # 1/sigma
            s_f = feat.tile([128, n_pad], f32r)      # sigma + mu^2
            w_f = feat.tile([128, n_pad], f32r)      # mu/sigma
            qcol = feat.tile([1, n_pad], f32r)       # q_j + pen_j
            w2a = feat.tile([128, n_anchor], f32r)   # -2*mu/sigma (anchors)
            m2a = feat.tile([128, n_anchor], f32r)   # -2*mu       (anchors)
            num_sl = accp.tile([128, n_it, n_groups], f32)
            den_sl = accp.tile([128, n_it, n_groups], f32)
            nd = accp.tile([128, n_it, 2], f32)

            pu_g = [None] * n_groups                 # mu^2/sigma, consumed by q matmul

            def phase1(g):
                gs = slice(g * GRP, (g + 1) * GRP)
                ps_mu = psMZ.tile([128, GRP], f32, tag="mu")
                ps_z = psMZ.tile([128, GRP], f32, tag="z")
                # sigma-head bias as a rank-1 matmul term
                nc.tensor.matmul(ps_z[:], bsigr_t[:], ones_r[:], start=True, stop=False)
                for k in range(6):
                    nc.tensor.matmul(ps_mu[:], wmu_t[:, k, :], h_t[:, k, gs],
                                     start=(k == 0), stop=(k == 5))
                    nc.tensor.matmul(ps_z[:], wsig_t[:, k, :], h_t[:, k, gs],
                                     start=False, stop=(k == 5))

                nc.vector.tensor_scalar_add(m_f[:, gs], ps_mu[:], bmuc_t[:])
                zm = work.tile([128, GRP], f32, tag="zm")
                nc.vector.tensor_scalar_min(zm[:], ps_z[:], 0.0)
                e1 = work.tile([128, GRP], f32, tag="e1")
                nc.scalar.activation(e1[:], zm[:], AF.Exp)
                # sigma = max(z,0) + exp(min(z,0))   (elu(z)+1)
                sig = work.tile([128, GRP], f32, tag="sig")
                nc.vector.scalar_tensor_tensor(sig[:], ps_z[:], 0.0, e1[:],
                                               ALU.max, ALU.add)
                with nc.allow_low_precision(reason="f32r out, full fp32 width"):
                    nc.vector.reciprocal(u_f[:, gs], sig[:])
                psq = work.tile([128, GRP], f32, tag="psq")
                nc.vector.tensor_mul(psq[:], m_f[:, gs].bitcast(f32),
                                     m_f[:, gs].bitcast(f32))
                nc.vector.tensor_add(s_f[:, gs], psq[:], sig[:])
                nc.vector.tensor_mul(w_f[:, gs], m_f[:, gs].bitcast(f32),
                                     u_f[:, gs].bitcast(f32))
                pu = work.tile([128, GRP], f32r, tag="pu")
                nc.vector.tensor_mul(pu[:], psq[:], u_f[:, gs].bitcast(f32))
                pu_g[g] = pu

            def qmm(g):
                gs = slice(g * GRP, (g + 1) * GRP)
                ps_q = psQ.tile([1, GRP], f32, tag="q")
                nc.tensor.matmul(ps_q[:], onescol_r[:], pu_g[g][:], start=True, stop=True)
                nc.vector.tensor_add(qcol[:, gs], ps_q[:], pen_t[:, gs].bitcast(f32))

            def anchors():
                nc.vector.tensor_scalar_mul(w2a[:], w_f[:, 0:n_anchor].bitcast(f32), -2.0)
                nc.vector.tensor_scalar_mul(m2a[:], m_f[:, 0:n_anchor].bitcast(f32), -2.0)

            def ph2(jt):
                jsl = slice(jt * GRP, (jt + 1) * GRP)
                ps2s = []
                exp1s = []
                for it, w in enumerate(it_ws):
                    isl = slice(it * 128, it * 128 + w)
                    ps2 = psP.tile([128, GRP], f32, tag="p2")
                    ps2s.append(ps2)
                    nc.tensor.matmul(ps2[:w], s_f[:, isl], u_f[:, jsl],
                                     start=True, stop=False)
                    nc.tensor.matmul(ps2[:w], u_f[:, isl], s_f[:, jsl],
                                     start=False, stop=False)
                    nc.tensor.matmul(ps2[:w], w2a[:, isl], m_f[:, jsl],
                                     start=False, stop=False)
                    nc.tensor.matmul(ps2[:w], m2a[:, isl], w_f[:, jsl],
                                     start=False, stop=False)
                    diag = (jt == 0)
                    nc.tensor.matmul(ps2[:w], ones_r[:, 0:w], qcol[:, jsl],
                                     start=False, stop=not diag)
                    if diag:
                        doff = it * 128
                        nc.tensor.matmul(ps2[:w, doff:doff + w],
                                         eqd_r[:, 0:w], eqd_r[:, 0:w],
                                         start=False, stop=True)
                    e_t = eph.tile([128, GRP], f32, tag="E")
                    exp1 = nc.scalar.activation(e_t[:w], ps2[:w], AF.Exp, scale=-0.25,
                                                bias=cbias_t[:w],
                                                accum_out=den_sl[:w, it, jt:jt + 1])
                    exp1s.append(exp1)
                # second pass: add -4*BIG*(1-labq) into ps2, exp again -> num
                for it, w in enumerate(it_ws):
                    isl = slice(it * 128, it * 128 + w)
                    ps2 = ps2s[it]
                    pen_mm = nc.tensor.matmul(ps2[:w], rowoh_t[:, isl], coloh_t[:, jsl],
                                              start=False, stop=True, skip_group_check=True)
                    add_dep_helper(pen_mm.ins, exp1s[it].ins, sync=True,
                                   reason="labq penalty matmul must follow den exp read")
                    e2 = eph.tile([128, GRP], f32, tag="E2")
                    nc.scalar.activation(e2[:w], ps2[:w], AF.Exp, scale=-0.25,
                                         bias=cbias_t[:w],
                                         accum_out=num_sl[:w, it, jt:jt + 1])

            phase1(0)
            anchors()
            for g in range(1, n_groups):
                phase1(g)
                qmm(g - 1)
                ph2(g - 1)
            qmm(n_groups - 1)
            ph2(n_groups - 1)

            for it, w in enumerate(it_ws):
                nc.vector.tensor_reduce(nd[:w, it, 0:1], num_sl[:w, it, :], AX.X, ALU.add)
                nc.vector.tensor_reduce(nd[:w, it, 1:2], den_sl[:w, it, :], AX.X, ALU.add)
                nc.sync.dma_start(out_d[it * 128:it * 128 + w, :], nd[0:w, it, :])

    nc.compile()
    return nc


def _prepare(ent_embeddings, ent_type_ids, ent_mask, W_mu, b_mu, W_sigma, b_sigma):
    emb = np.ascontiguousarray(np.asarray(ent_embeddings, dtype=np.float32)).reshape(-1, H_DIM)
    labels = np.asarray(ent_type_ids).reshape(-1).astype(np.int64)
    mask = np.asarray(ent_mask).reshape(-1).astype(np.int64)
    W_mu = np.ascontiguousarray(np.asarray(W_mu, dtype=np.float32))
    W_sigma = np.ascontiguousarray(np.asarray(W_sigma, dtype=np.float32))
    b_mu = np.ascontiguousarray(np.asarray(b_mu, dtype=np.float32))
    b_sigma = np.ascontiguousarray(np.asarray(b_sigma, dtype=np.float32))

    valid = (mask == 1) & (labels >= 0)
    vidx = np.nonzero(valid)[0]
    n_v = len(vidx)
    if n_v == 0:
        return None

    n_pad = GRP * max(1, -(-n_v // GRP))
    n_anchor = n_pad // 8
    labs = labels[vidx]
    assert labs.max() < NCLS, "labels exceed one-hot width"

    import ml_dtypes
    bf16 = ml_dtypes.bfloat16
    h = np.zeros((n_pad, H_DIM), dtype=np.float32)
    h[:n_v] = np.maximum(emb[vidx], 0.0)
    hT = np.ascontiguousarray(h.T).astype(bf16)         # [768, n_pad]
    labc = np.full(n_pad, -1, dtype=np.int64)
    labc[:n_v] = labs
    pen = np.full(n_pad, BIG, dtype=np.float32)
    pen[:n_v] = 0.0
    eqd = (np.eye(128, dtype=np.float32) * SQRT_BIG)
    cls = np.arange(NCLS, dtype=np.int64)

    if n_pad not in _cache:
        _cache[n_pad] = _build(n_pad)
    nc = _cache[n_pad]

    in_maps = []
    for c in range(8):
        r = c * n_anchor
        labr = np.roll(labc, -r)
        # penalty matmul operands: sum_c rowP[c,i]*colP[c,j] = 4*BIG*(1-labq)
        rowP = np.empty((NCLS + 1, n_anchor), dtype=np.float32)
        rowP[:NCLS] = -2.0 * SQRT_BIG * (labr[None, :n_anchor] == cls[:, None])
        rowP[NCLS] = 2.0 * SQRT_BIG
        colP = np.empty((NCLS + 1, n_pad), dtype=np.float32)
        colP[:NCLS] = 2.0 * SQRT_BIG * (labr[None, :] == cls[:, None])
        colP[NCLS] = 2.0 * SQRT_BIG
        in_maps.append({
            "ht": np.ascontiguousarray(np.roll(hT, -r, axis=1)),
            "wmu": W_mu.astype(bf16), "wsig": W_sigma.astype(bf16),
            "bmu": b_mu, "bsig": b_sigma,
            "pen": np.roll(pen, -r),
            "rowoh": rowP,
            "coloh": colP,
            "eqd": eqd,
        })
    return dict(nc=nc, in_maps=in_maps, n_pad=n_pad, n_anchor=n_anchor,
                n_v=n_v, labs=labs)


def _finish(prep, res):
    n_pad, n_anchor, n_v = prep["n_pad"], prep["n_anchor"], prep["n_v"]
    num = np.empty(n_pad, dtype=np.float32)
    den = np.empty(n_pad, dtype=np.float32)
    for c in range(8):
        nd = res.results[c]["out"]
        rows = (np.arange(n_anchor) + c * n_anchor) % n_pad
        num[rows] = nd[:, 0]
        den[rows] = nd[:, 1]

    labs = prep["labs"]
    hist = np.bincount(labs, minlength=int(labs.max()) + 1)
    cnt = (hist[labs] - 1).astype(np.float64)
    sel = cnt > 0
    n_sel = max(sel.sum(), 1)
    num_v = num[:n_v].astype(np.float64)
    den_v = den[:n_v].astype(np.float64)
    safe_num = np.where(sel, num_v, 1.0)
    safe_den = np.where(sel, den_v, 1.0)
    safe_cnt = np.where(sel, cnt, 1.0)
    lf = (np.log(safe_den) - np.log(safe_num)) / np.log(2.0) + np.log2(safe_cnt)
    total = np.sum(np.where(sel, lf, 0.0)) / n_sel
    return np.float32(total)


def kernel(ent_embeddings, ent_type_ids, ent_mask, W_mu, b_mu, W_sigma, b_sigma):
    from concourse.bass_utils import run_bass_kernel_spmd

    prep = _prepare(ent_embeddings, ent_type_ids, ent_mask,
                    W_mu, b_mu, W_sigma, b_sigma)
    if prep is None:
        return np.float32(0.0)
    res = run_bass_kernel_spmd(prep["nc"], prep["in_maps"], list(range(8)))
    return _finish(prep, res)
